# revision 16
# baseline (speedup 1.0000x reference)
"""Trainium2 Bass kernel for column self-attention (nn_ColumnSelfAttention).

Reference computation (per column c, columns are independent attention
problems):
    q = (x @ Wq + bq) * head_dim**-0.5 ; k = x @ Wk + bk ; v = x @ Wv + bv
    scores[h,c,i,j] = sum_d q[i,c,h,d] k[j,c,h,d]
    scores = where(mask[j,c], scores, -1e4); p = softmax_j(scores)
    ctx[i,c,:] = concat_h(p @ v) ; out = ctx @ Wo + bo

Sharding: the 256 columns are split across 8 NeuronCores (32 each).
Per core, tokens are ordered (column-major, row-inner) so one 128-token
tile == one column.  All matmul inputs are fp16 (fp32 PSUM accumulation);
softmax runs in fp32 on the scalar/vector engines.

Layout trick: scores are computed transposed (S_T[j,i]) so the key mask
becomes a per-partition bias fused into the Exp activation, and the
softmax denominator comes for free from an appended ones-column on V
(out[:, 64] of each head block = row sums).  The only transpose needed
is x (host-side) and the normalized context (PE transpose via identity
matmul) before the output projection.

Perf notes (vs the 400us/336us baseline):
  - weights are host-flattened to [128, NCH*E] so each is ONE contiguous
    HWDGE DMA on the sync queue (the old strided gather on the SWDGE
    queue gated the first matmul to t=16.3us); x is host-blocked to
    [NBLK, 128, NCH*T], block 0 split in two halves on the scalar queue
    (a single HWDGE queue only sustains ~90-180 GB/s, so splitting the
    critical first loads across both queues matters).
  - qz (zero-padded Q) and va (V+ones) live in persistent double
    buffers, memset ONCE (the zero/one regions are never overwritten),
    not per block: GpSimd -38us busy.
  - psmm PSUM pool bufs 2->3 (WO matmuls no longer stall ~290ns on the
    scalar PSUM->SBUF drains); pscx 3->2 to stay within 8 banks.
  - last column's output DMA goes on sync (idle by then) instead of the
    SWDGE queue, shortening the tail drain.
  - DMA xbar transposes of ctx and per-column output DMAs on sync were
    tried and REVERTED: each DMA_TRANSPOSE costs ~1.3us of sync
    sequencer time and the rotating DMA-completion semaphores couple
    the transpose path to slow SWDGE output transfers, stalling WO
    matmuls 4-5us per block (411us / 394us vs 336us baseline).
"""

import os
import numpy as np

import concourse.bacc as bacc
import concourse.tile as tile
import concourse.mybir as mybir
from concourse import bass
from concourse.bass_utils import run_bass_kernel_spmd

R, C, E, H, D = 128, 256, 768, 12, 64
NCORES = 8
CLOC = C // NCORES            # 32 columns per core
BLK = 4                       # columns per block
NBLK = CLOC // BLK
T = BLK * R                   # 512 tokens per block
NTOK = CLOC * R               # 4096 tokens per core
NCH = E // 128                # 6 chunks of the embedding dim
F16 = mybir.dt.float16
F32 = mybir.dt.float32
Act = mybir.ActivationFunctionType

LAST_RESULTS = None           # for test.py introspection


def build_program(with_bias: bool, nblocks: int = NBLK, stage: int = 8):
    nc = bacc.Bacc("TRN2", target_bir_lowering=False, debug=False)

    # x^T per core, host-pretransposed AND host-blocked:
    # x_d[b, p, c*T+t] = x^T[c*128+p, b*512+t] -- one contiguous DMA/block
    x_d = nc.dram_tensor("x", [NBLK, 128, NCH * T], F16, kind="ExternalInput")
    madd_d = nc.dram_tensor("madd", [R, CLOC], F32, kind="ExternalInput")
    ident_d = nc.dram_tensor("ident", [128, 128], F16, kind="ExternalInput")
    # weights host-flattened: w_d[p, c*E+e] = W[c*128+p, e] -- contiguous
    w_d = {
        n: nc.dram_tensor(n, [128, NCH * E], F16, kind="ExternalInput")
        for n in ("wq", "wk", "wv", "wo")
    }
    if with_bias:
        bqk_d = {
            n: nc.dram_tensor(n, [128, NCH], F32, kind="ExternalInput")
            for n in ("bq", "bk")
        }
        bvo_d = {
            n: nc.dram_tensor(n, [1, E], F16, kind="ExternalInput")
            for n in ("bv", "bo")
        }
    o_d = nc.dram_tensor("o", [nblocks * BLK, R, E], F32, kind="ExternalOutput")

    with tile.TileContext(nc) as tc:
        with (
            tc.tile_pool(name="const", bufs=1) as const,
            tc.tile_pool(name="blk", bufs=2) as blkp,
            tc.tile_pool(name="col", bufs=4) as colp,
            tc.tile_pool(name="psmm", bufs=3, space="PSUM") as psmm,
            tc.tile_pool(name="pss", bufs=3, space="PSUM") as pssp,
            tc.tile_pool(name="pscx", bufs=2, space="PSUM") as pscx,
        ):
            w_sb = {}
            for n in ("wq", "wk", "wv", "wo"):
                w_sb[n] = const.tile([128, NCH, E], F16, tag=n, name=f"w_{n}")
            madd_sb = const.tile([R, CLOC], F32, tag="madd")
            ident_sb = const.tile([128, 128], F16, tag="ident")
            # persistent double buffers: the zero rows of qz / ones cols of
            # va are written once here and never overwritten by the loop
            qzp = [const.tile([128, H, T], F16, tag=f"qz{i}", name=f"qz{i}")
                   for i in range(2)]
            # va is per-COLUMN tiles: tile-granular dependency tracking
            # otherwise makes PV(0) wait for the LAST column's V cast
            vap = [[const.tile([128, H * 65], F16, tag=f"va{i}_{t}",
                               name=f"va{i}_{t}") for t in range(BLK)]
                   for i in range(2)]

            # head loading: a single DMA queue only sustains ~105 GB/s, so
            # the first-needed bytes (wq + block-0 x) are chunk-paired
            # across all three queues in the PE's consumption order
            # (Q-proj co=0 consumes (wq_k, xt_k) for k=0..5)
            xt0 = blkp.tile([128, NCH, T], F16, tag="xt", name="xt0")
            x0flat = xt0.rearrange("p c t -> p (c t)")
            qdma = [nc.sync, nc.scalar, nc.gpsimd]
            for k in range(NCH):
                eng = qdma[k % 3]
                eng.dma_start(
                    w_sb["wq"][:, k, :], w_d["wq"].ap()[:, k * E : (k + 1) * E]
                )
                eng.dma_start(
                    x0flat[:, k * T : (k + 1) * T],
                    x_d.ap()[0, :, k * T : (k + 1) * T],
                )
            # remaining weights: halves on sync+scalar, in consumption order
            for n in ("wk", "wv", "wo"):
                wf = w_sb[n].rearrange("p c e -> p (c e)")
                hw = NCH * E // 2
                nc.sync.dma_start(wf[:, 0:hw], w_d[n].ap()[:, 0:hw])
                nc.scalar.dma_start(wf[:, hw:], w_d[n].ap()[:, hw:])
            # gpsimd: small loads, then block-1's one-time memsets
            nc.gpsimd.dma_start(madd_sb[:], madd_d.ap())
            nc.gpsimd.dma_start(ident_sb[:], ident_d.ap())
            if with_bias:
                bqk_sb = {}
                for n in ("bq", "bk"):
                    bqk_sb[n] = const.tile([128, NCH], F32, tag=n, name=f"b_{n}")
                    nc.gpsimd.dma_start(bqk_sb[n][:], bqk_d[n].ap())
                bvo_sb = {}
                for n in ("bv", "bo"):
                    bvo_sb[n] = const.tile([1, E], F16, tag=n, name=f"b_{n}")
                    nc.gpsimd.dma_start(bvo_sb[n][:], bvo_d[n].ap())
                ones_sb = const.tile([1, 128], F16, tag="ones")
                nc.gpsimd.memset(ones_sb[:], 1.0)
            # block-0's qz/va memsets on the (idle) vector engine so they
            # are ready before the first qz copies at ~10us
            nc.vector.memset(qzp[0][:], 0.0)
            for t in range(BLK):
                nc.vector.memset(vap[0][t][:], 1.0)
            nc.gpsimd.memset(qzp[1][:], 0.0)
            for t in range(BLK):
                nc.gpsimd.memset(vap[1][t][:], 1.0)

            pending_wo = None
            for b in range(nblocks):
                qz = qzp[b % 2]
                va = vap[b % 2]
                # ---- x^T for this block: one contiguous DMA ----
                if b == 0:
                    xt = xt0
                else:
                    xt = blkp.tile([128, NCH, T], F16, tag="xt")
                    nc.scalar.dma_start(
                        xt.rearrange("p c t -> p (c t)"), x_d.ap()[b]
                    )

                if stage < 2:
                    continue
                # ---- Q^T, K^T projections: (e_out, tok) ----
                qt = blkp.tile([128, NCH, T], F16, tag="qt")
                kt = blkp.tile([128, NCH, T], F16, tag="kt")
                for wname, bname, dst in (("wq", "bq", qt), ("wk", "bk", kt)):
                    for co in range(NCH):
                        ps = psmm.tile([128, T], F32, tag="mm")
                        for k in range(NCH):
                            nc.tensor.matmul(
                                ps[:],
                                w_sb[wname][:, k, co * 128 : (co + 1) * 128],
                                xt[:, k, :],
                                start=(k == 0),
                                stop=(k == NCH - 1),
                            )
                        if with_bias:
                            nc.scalar.activation(
                                dst[:, co, :], ps[:], Act.Identity,
                                bias=bqk_sb[bname][:, co : co + 1],
                            )
                        elif wname == "wk":
                            # K copies on vector: scalar alone (Q copies +
                            # exp backlog) fell behind the PE during the QK
                            # phase and stalled matmuls on psmm WAR
                            nc.vector.tensor_copy(dst[:, co, :], ps[:])
                        else:
                            nc.scalar.copy(dst[:, co, :], ps[:])
                        if stage >= 3 and wname == "wq":
                            # per-head zero-padded Q^T, emitted per-chunk so
                            # the DVE copies spread across the QK phase
                            # instead of bursting at block start
                            # (base-partition-64 matmuls into shared PSUM
                            # banks crash HW; scores contract K=128 instead,
                            # with the other head's rows zeroed on the Q side)
                            for h in (2 * co, 2 * co + 1):
                                off = (h % 2) * 64
                                nc.vector.tensor_copy(
                                    qz[off : off + 64, h, :],
                                    qt[off : off + 64, co, :],
                                )

                if stage < 3:
                    continue

                # ---- V projection, natural layout, interleaved with a ones
                # column per head: va[:, t, h*65:h*65+64] = V_h, [...,64] = 1 ----
                for t in range(BLK):
                    for half in range(2):
                        psv = psmm.tile([128, 384], F32, tag="mm")
                        if with_bias:
                            nc.tensor.matmul(
                                psv[:], ones_sb[:],
                                bvo_sb["bv"][:, half * 384 : (half + 1) * 384],
                                start=True, stop=False,
                            )
                        for k in range(NCH):
                            nc.tensor.matmul(
                                psv[:],
                                xt[:, k, t * 128 : (t + 1) * 128],
                                w_sb["wv"][:, k, half * 384 : (half + 1) * 384],
                                start=(k == 0 and not with_bias),
                                stop=(k == NCH - 1),
                            )
                        dst = va[t][:, half * 390 : (half + 1) * 390]
                        dst = dst.rearrange("p (h x) -> p h x", x=65)[:, :, 0:64]
                        nc.vector.tensor_copy(
                            dst, psv[:].rearrange("p (h d) -> p h d", d=64)
                        )

                # ---- attention, software-pipelined across columns so the
                # PE always has independent work while the per-column
                # PV -> recip/normalize (DVE) -> transpose -> copy -> Wo
                # chain drains.  PE emission order per cycle t:
                #   PV(t), S(t+2), WO(t-1), TR(t)
                if stage < 4:
                    continue
                ets, pscs, ctxnts = {}, {}, {}

                def emit_scores(t):
                    cg = b * BLK + t
                    et = colp.tile([128, H * 128], F16, tag="et",
                                   name=f"et_{b}_{t}")
                    for g3 in range(3):
                        pss = pssp.tile([128, 512], F32, tag="s", name="pss")
                        for pp in range(2):
                            # head pair (2hp, 2hp+1) shares the kt chunk hp
                            # as stationary: one 256-stream matmul per pair
                            hp = g3 * 2 + pp
                            nc.tensor.matmul(
                                pss[:, pp * 256 : (pp + 1) * 256],
                                kt[:, hp, t * 128 : (t + 1) * 128],
                                qz[:, 2 * hp : 2 * hp + 2,
                                   t * 128 : (t + 1) * 128],
                                start=(pp == 0),
                                stop=(pp == 1),
                            )
                        nc.scalar.activation(
                            et[:, g3 * 512 : (g3 + 1) * 512], pss[:], Act.Exp,
                            bias=madd_sb[:, cg : cg + 1], scale=1.0,
                        )
                    ets[t] = et

                def emit_pv(t):
                    if stage < 5:
                        return
                    et = ets[t]
                    psc = []
                    for g2 in range(2):
                        pc = pscx.tile([128, 390], F32, tag="cx", name="pc")
                        for hh in range(6):
                            h = g2 * 6 + hh
                            nc.tensor.matmul(
                                pc[:, hh * 65 : (hh + 1) * 65],
                                et[:, h * 128 : (h + 1) * 128],
                                va[t][:, h * 65 : (h + 1) * 65],
                                start=(hh == 0),
                                stop=(hh == 5),
                            )
                        psc.append(pc)
                    pscs[t] = psc

                def emit_norm_tr(t):
                    if stage < 6:
                        return
                    psc = pscs[t]
                    recip = colp.tile([128, H], F32, tag="recip", name="recip")
                    ctxn = colp.tile([128, E], F16, tag="ctxn", name="ctxn")
                    for g2 in range(2):
                        grp = psc[g2].rearrange("p (h x) -> p h x", x=65)
                        nc.vector.reciprocal(
                            recip[:, g2 * 6 : (g2 + 1) * 6].unsqueeze(2),
                            grp[:, :, 64:65],
                        )
                        nc.vector.tensor_mul(
                            ctxn[:, g2 * 384 : (g2 + 1) * 384].rearrange(
                                "p (h d) -> p h d", d=64
                            ),
                            grp[:, :, 0:64],
                            recip[:, g2 * 6 : (g2 + 1) * 6]
                            .unsqueeze(2)
                            .broadcast_to((128, 6, 64)),
                        )
                    if stage < 7:
                        return
                    pst = pscx.tile([128, NCH, 128], F16, tag="cx", name="pst")
                    for ec in range(NCH):
                        nc.tensor.transpose(
                            pst[:, ec, :],
                            ctxn[:, ec * 128 : (ec + 1) * 128],
                            ident_sb[:],
                        )
                    ctxnt = colp.tile([128, NCH, 128], F16, tag="ctxnt",
                                      name="ctxnt")
                    nc.vector.tensor_copy(ctxnt[:], pst[:])
                    ctxnts[t] = ctxnt

                def emit_wo(t, cg, store):
                    if stage < 8 or t not in store:
                        return
                    ctxnt = store.pop(t)
                    osb = colp.tile([128, E], F32, tag="osb", name="osb")
                    for half in range(2):
                        po = psmm.tile([128, 384], F32, tag="mm", name="po")
                        if with_bias:
                            nc.tensor.matmul(
                                po[:], ones_sb[:],
                                bvo_sb["bo"][:, half * 384 : (half + 1) * 384],
                                start=True, stop=False,
                            )
                        for k in range(NCH):
                            nc.tensor.matmul(
                                po[:],
                                ctxnt[:, k, :],
                                w_sb["wo"][:, k, half * 384 : (half + 1) * 384],
                                start=(k == 0 and not with_bias),
                                stop=(k == NCH - 1),
                            )
                        nc.scalar.copy(osb[:, half * 384 : (half + 1) * 384], po[:])
                    # all outputs on sync: it is ~idle, HWDGE transfers
                    # finish in ~1.1us, and the tail avoids the 3us SWDGE
                    # output-queue drain
                    nc.sync.dma_start(o_d.ap()[cg], osb[:])

                emit_scores(0)
                if BLK > 1:
                    emit_scores(1)
                for t in range(BLK):
                    emit_pv(t)
                    if t + 2 < BLK:
                        emit_scores(t + 2)
                    if t >= 1:
                        emit_wo(t - 1, b * BLK + t - 1, ctxnts)
                    elif pending_wo is not None:
                        pending_wo()           # last column of previous block
                        pending_wo = None
                    emit_norm_tr(t)
                import functools
                pending_wo = functools.partial(
                    emit_wo, BLK - 1, b * BLK + BLK - 1, ctxnts
                )

            if pending_wo is not None:
                pending_wo()
    nc.compile()
    return nc


_PROGRAMS = {}


def _get_program(with_bias: bool):
    if with_bias not in _PROGRAMS:
        _PROGRAMS[with_bias] = build_program(with_bias)
    return _PROGRAMS[with_bias]


def make_in_maps(x, self_attn_padding_mask, Wq, bq, Wk, bk, Wv, bv, Wo, bo,
                 with_bias):
    scaling = float(D) ** -0.5
    # host-flattened weights: [128, NCH*E] with w[p, c*E+e] = W[c*128+p, e]
    def wflat(W, scale=1.0):
        w = (np.asarray(W, np.float32) * scale).astype(np.float16)
        return np.ascontiguousarray(
            w.reshape(NCH, 128, E).transpose(1, 0, 2).reshape(128, NCH * E)
        )

    wq = wflat(Wq, scaling)
    wk = wflat(Wk)
    wv = wflat(Wv)
    wo = wflat(Wo)
    mask = np.asarray(self_attn_padding_mask)[0]                   # (R, C)
    madd_full = np.where(mask, 0.0, -10000.0).astype(np.float32)   # (R, C)
    xf = np.asarray(x, np.float32)[:, :, 0, :]                     # (R, C, E)
    ident = np.eye(128, dtype=np.float16)
    in_maps = []
    for i in range(NCORES):
        cs = slice(i * CLOC, (i + 1) * CLOC)
        xs = (
            xf[:, cs]
            .transpose(1, 0, 2)                # (CLOC, R, E) tok-major
            .reshape(NTOK, NCH, 128)
            .transpose(1, 2, 0)                # (NCH, 128, NTOK) = x^T chunks
        )
        # block-major: x_b[b, p, c*T+t] = xs[c, p, b*T+t]
        xb = (
            xs.reshape(NCH, 128, NBLK, T)
            .transpose(2, 1, 0, 3)
            .reshape(NBLK, 128, NCH * T)
        )
        xb = np.ascontiguousarray(xb.astype(np.float16))
        m = {
            "x": xb,
            "madd": np.ascontiguousarray(madd_full[:, cs]),
            "wq": wq, "wk": wk, "wv": wv, "wo": wo,
            "ident": ident,
        }
        if with_bias:
            m["bq"] = np.ascontiguousarray(
                (np.asarray(bq, np.float32) * scaling).reshape(NCH, 128).T
            )
            m["bk"] = np.ascontiguousarray(
                np.asarray(bk, np.float32).reshape(NCH, 128).T
            )
            m["bv"] = np.asarray(bv, np.float32).astype(np.float16).reshape(1, E)
            m["bo"] = np.asarray(bo, np.float32).astype(np.float16).reshape(1, E)
        in_maps.append(m)
    return in_maps


def assemble_output(shards):
    out = np.empty((R, C, 1, E), np.float32)
    for i in range(NCORES):
        out[:, i * CLOC : (i + 1) * CLOC, 0, :] = shards[i].transpose(1, 0, 2)
    return out


def kernel(x, self_attn_padding_mask, Wq, bq, Wk, bk, Wv, bv, Wo, bo):
    global LAST_RESULTS
    with_bias = any(
        bool(np.any(np.asarray(b))) for b in (bq, bk, bv, bo)
    )
    nc = _get_program(with_bias)
    in_maps = make_in_maps(
        x, self_attn_padding_mask, Wq, bq, Wk, bk, Wv, bv, Wo, bo, with_bias
    )
    trace = os.environ.get("KERNEL_TRACE", "") not in ("", "0")
    res = run_bass_kernel_spmd(
        nc, in_maps, core_ids=list(range(NCORES)), trace=trace
    )
    LAST_RESULTS = res
    return assemble_output([res.results[i]["o"] for i in range(NCORES)])


# revision 19
# speedup vs baseline: 1.0029x; 1.0029x over previous
"""Trainium2 Bass kernel for column self-attention (nn_ColumnSelfAttention).

Reference computation (per column c, columns are independent attention
problems):
    q = (x @ Wq + bq) * head_dim**-0.5 ; k = x @ Wk + bk ; v = x @ Wv + bv
    scores[h,c,i,j] = sum_d q[i,c,h,d] k[j,c,h,d]
    scores = where(mask[j,c], scores, -1e4); p = softmax_j(scores)
    ctx[i,c,:] = concat_h(p @ v) ; out = ctx @ Wo + bo

Sharding: the 256 columns are split across 8 NeuronCores (32 each).
Per core, tokens are ordered (column-major, row-inner) so one 128-token
tile == one column.  All matmul inputs are fp16 (fp32 PSUM accumulation);
softmax runs in fp32 on the scalar/vector engines.

Layout trick: scores are computed transposed (S_T[j,i]) so the key mask
becomes a per-partition bias fused into the Exp activation, and the
softmax denominator comes for free from an appended ones-column on V
(out[:, 64] of each head block = row sums).  The only transpose needed
is x (host-side) and the normalized context (PE transpose via identity
matmul) before the output projection.

Perf notes (vs the 400us/336us baseline):
  - weights are host-flattened to [128, NCH*E] so each is ONE contiguous
    HWDGE DMA on the sync queue (the old strided gather on the SWDGE
    queue gated the first matmul to t=16.3us); x is host-blocked to
    [NBLK, 128, NCH*T], block 0 split in two halves on the scalar queue
    (a single HWDGE queue only sustains ~90-180 GB/s, so splitting the
    critical first loads across both queues matters).
  - qz (zero-padded Q) and va (V+ones) live in persistent double
    buffers, memset ONCE (the zero/one regions are never overwritten),
    not per block: GpSimd -38us busy.
  - psmm PSUM pool bufs 2->3 (WO matmuls no longer stall ~290ns on the
    scalar PSUM->SBUF drains); pscx 3->2 to stay within 8 banks.
  - last column's output DMA goes on sync (idle by then) instead of the
    SWDGE queue, shortening the tail drain.
  - DMA xbar transposes of ctx and per-column output DMAs on sync were
    tried and REVERTED: each DMA_TRANSPOSE costs ~1.3us of sync
    sequencer time and the rotating DMA-completion semaphores couple
    the transpose path to slow SWDGE output transfers, stalling WO
    matmuls 4-5us per block (411us / 394us vs 336us baseline).
"""

import os
import numpy as np

import concourse.bacc as bacc
import concourse.tile as tile
import concourse.mybir as mybir
from concourse import bass
from concourse.bass_utils import run_bass_kernel_spmd

R, C, E, H, D = 128, 256, 768, 12, 64
NCORES = 8
CLOC = C // NCORES            # 32 columns per core
BLK = 4                       # columns per block
NBLK = CLOC // BLK
T = BLK * R                   # 512 tokens per block
NTOK = CLOC * R               # 4096 tokens per core
NCH = E // 128                # 6 chunks of the embedding dim
F16 = mybir.dt.float16
F32 = mybir.dt.float32
Act = mybir.ActivationFunctionType

LAST_RESULTS = None           # for test.py introspection


def build_program(with_bias: bool, nblocks: int = NBLK, stage: int = 8):
    nc = bacc.Bacc("TRN2", target_bir_lowering=False, debug=False)

    # x^T per core, host-pretransposed AND host-blocked:
    # x_d[b, p, c*T+t] = x^T[c*128+p, b*512+t] -- one contiguous DMA/block
    x_d = nc.dram_tensor("x", [NBLK, 128, NCH * T], F16, kind="ExternalInput")
    madd_d = nc.dram_tensor("madd", [R, CLOC], F32, kind="ExternalInput")
    ident_d = nc.dram_tensor("ident", [128, 128], F16, kind="ExternalInput")
    # weights host-flattened: w_d[p, c*E+e] = W[c*128+p, e] -- contiguous
    w_d = {
        n: nc.dram_tensor(n, [128, NCH * E], F16, kind="ExternalInput")
        for n in ("wq", "wk", "wv", "wo")
    }
    if with_bias:
        bqk_d = {
            n: nc.dram_tensor(n, [128, NCH], F32, kind="ExternalInput")
            for n in ("bq", "bk")
        }
        bvo_d = {
            n: nc.dram_tensor(n, [1, E], F16, kind="ExternalInput")
            for n in ("bv", "bo")
        }
    o_d = nc.dram_tensor("o", [nblocks * BLK, R, E], F32, kind="ExternalOutput")

    with tile.TileContext(nc) as tc:
        with (
            tc.tile_pool(name="const", bufs=1) as const,
            tc.tile_pool(name="blk", bufs=2) as blkp,
            tc.tile_pool(name="col", bufs=4) as colp,
            tc.tile_pool(name="psmm", bufs=3, space="PSUM") as psmm,
            tc.tile_pool(name="pss", bufs=3, space="PSUM") as pssp,
            tc.tile_pool(name="pscx", bufs=2, space="PSUM") as pscx,
        ):
            w_sb = {}
            for n in ("wq", "wk", "wv", "wo"):
                w_sb[n] = const.tile([128, NCH, E], F16, tag=n, name=f"w_{n}")
            madd_sb = const.tile([R, CLOC], F32, tag="madd")
            ident_sb = const.tile([128, 128], F16, tag="ident")
            # persistent double buffers: the zero rows of qz / ones cols of
            # va are written once here and never overwritten by the loop
            qzp = [const.tile([128, H, T], F16, tag=f"qz{i}", name=f"qz{i}")
                   for i in range(2)]
            vap = [const.tile([128, BLK, H * 65], F16, tag=f"va{i}",
                              name=f"va{i}") for i in range(2)]

            # head loading: a single DMA queue only sustains ~105 GB/s, so
            # the first-needed bytes (wq + block-0 x) are chunk-paired
            # across all three queues in the PE's consumption order
            # (Q-proj co=0 consumes (wq_k, xt_k) for k=0..5)
            xt0 = blkp.tile([128, NCH, T], F16, tag="xt", name="xt0")
            x0flat = xt0.rearrange("p c t -> p (c t)")
            qdma = [nc.sync, nc.scalar, nc.gpsimd]
            for k in range(NCH):
                eng = qdma[k % 3]
                eng.dma_start(
                    w_sb["wq"][:, k, :], w_d["wq"].ap()[:, k * E : (k + 1) * E]
                )
                eng.dma_start(
                    x0flat[:, k * T : (k + 1) * T],
                    x_d.ap()[0, :, k * T : (k + 1) * T],
                )
            # remaining weights: halves on sync+scalar, in consumption order
            for n in ("wk", "wv", "wo"):
                wf = w_sb[n].rearrange("p c e -> p (c e)")
                hw = NCH * E // 2
                nc.sync.dma_start(wf[:, 0:hw], w_d[n].ap()[:, 0:hw])
                nc.scalar.dma_start(wf[:, hw:], w_d[n].ap()[:, hw:])
            # gpsimd: small loads, then block-1's one-time memsets
            nc.gpsimd.dma_start(madd_sb[:], madd_d.ap())
            nc.gpsimd.dma_start(ident_sb[:], ident_d.ap())
            if with_bias:
                bqk_sb = {}
                for n in ("bq", "bk"):
                    bqk_sb[n] = const.tile([128, NCH], F32, tag=n, name=f"b_{n}")
                    nc.gpsimd.dma_start(bqk_sb[n][:], bqk_d[n].ap())
                bvo_sb = {}
                for n in ("bv", "bo"):
                    bvo_sb[n] = const.tile([1, E], F16, tag=n, name=f"b_{n}")
                    nc.gpsimd.dma_start(bvo_sb[n][:], bvo_d[n].ap())
                ones_sb = const.tile([1, 128], F16, tag="ones")
                nc.gpsimd.memset(ones_sb[:], 1.0)
            # block-0's qz/va memsets on the (idle) vector engine so they
            # are ready before the first qz copies at ~10us
            nc.vector.memset(qzp[0][:], 0.0)
            nc.vector.memset(vap[0][:], 1.0)
            nc.gpsimd.memset(qzp[1][:], 0.0)
            nc.gpsimd.memset(vap[1][:], 1.0)

            pending_wo = None
            for b in range(nblocks):
                qz = qzp[b % 2]
                va = vap[b % 2]
                # ---- x^T for this block: one contiguous DMA ----
                if b == 0:
                    xt = xt0
                else:
                    xt = blkp.tile([128, NCH, T], F16, tag="xt")
                    nc.scalar.dma_start(
                        xt.rearrange("p c t -> p (c t)"), x_d.ap()[b]
                    )

                if stage < 2:
                    continue
                # ---- Q^T, K^T projections: (e_out, tok) ----
                qt = blkp.tile([128, NCH, T], F16, tag="qt")
                kt = blkp.tile([128, NCH, T], F16, tag="kt")
                for wname, bname, dst in (("wq", "bq", qt), ("wk", "bk", kt)):
                    for co in range(NCH):
                        ps = psmm.tile([128, T], F32, tag="mm")
                        for k in range(NCH):
                            nc.tensor.matmul(
                                ps[:],
                                w_sb[wname][:, k, co * 128 : (co + 1) * 128],
                                xt[:, k, :],
                                start=(k == 0),
                                stop=(k == NCH - 1),
                            )
                        if with_bias:
                            nc.scalar.activation(
                                dst[:, co, :], ps[:], Act.Identity,
                                bias=bqk_sb[bname][:, co : co + 1],
                            )
                        elif wname == "wk":
                            # K copies on vector: scalar alone (Q copies +
                            # exp backlog) fell behind the PE during the QK
                            # phase and stalled matmuls on psmm WAR
                            nc.vector.tensor_copy(dst[:, co, :], ps[:])
                        else:
                            nc.scalar.copy(dst[:, co, :], ps[:])
                        if stage >= 3 and wname == "wq":
                            # per-head zero-padded Q^T, emitted per-chunk so
                            # the DVE copies spread across the QK phase
                            # instead of bursting at block start
                            # (base-partition-64 matmuls into shared PSUM
                            # banks crash HW; scores contract K=128 instead,
                            # with the other head's rows zeroed on the Q side)
                            for h in (2 * co, 2 * co + 1):
                                off = (h % 2) * 64
                                nc.vector.tensor_copy(
                                    qz[off : off + 64, h, :],
                                    qt[off : off + 64, co, :],
                                )

                if stage < 3:
                    continue

                # ---- V projection, natural layout, interleaved with a ones
                # column per head: va[:, t, h*65:h*65+64] = V_h, [...,64] = 1 ----
                for t in range(BLK):
                    for half in range(2):
                        psv = psmm.tile([128, 384], F32, tag="mm")
                        if with_bias:
                            nc.tensor.matmul(
                                psv[:], ones_sb[:],
                                bvo_sb["bv"][:, half * 384 : (half + 1) * 384],
                                start=True, stop=False,
                            )
                        for k in range(NCH):
                            nc.tensor.matmul(
                                psv[:],
                                xt[:, k, t * 128 : (t + 1) * 128],
                                w_sb["wv"][:, k, half * 384 : (half + 1) * 384],
                                start=(k == 0 and not with_bias),
                                stop=(k == NCH - 1),
                            )
                        dst = va[:, t, half * 390 : (half + 1) * 390]
                        dst = dst.rearrange("p (h x) -> p h x", x=65)[:, :, 0:64]
                        nc.vector.tensor_copy(
                            dst, psv[:].rearrange("p (h d) -> p h d", d=64)
                        )

                # ---- attention, software-pipelined across columns so the
                # PE always has independent work while the per-column
                # PV -> recip/normalize (DVE) -> transpose -> copy -> Wo
                # chain drains.  PE emission order per cycle t:
                #   PV(t), S(t+2), WO(t-1), TR(t)
                if stage < 4:
                    continue
                ets, pscs, ctxnts = {}, {}, {}

                def emit_scores(t):
                    cg = b * BLK + t
                    et = colp.tile([128, H * 128], F16, tag="et",
                                   name=f"et_{b}_{t}")
                    for g3 in range(3):
                        pss = pssp.tile([128, 512], F32, tag="s", name="pss")
                        for pp in range(2):
                            # head pair (2hp, 2hp+1) shares the kt chunk hp
                            # as stationary: one 256-stream matmul per pair
                            hp = g3 * 2 + pp
                            nc.tensor.matmul(
                                pss[:, pp * 256 : (pp + 1) * 256],
                                kt[:, hp, t * 128 : (t + 1) * 128],
                                qz[:, 2 * hp : 2 * hp + 2,
                                   t * 128 : (t + 1) * 128],
                                start=(pp == 0),
                                stop=(pp == 1),
                            )
                        nc.scalar.activation(
                            et[:, g3 * 512 : (g3 + 1) * 512], pss[:], Act.Exp,
                            bias=madd_sb[:, cg : cg + 1], scale=1.0,
                        )
                    ets[t] = et

                def emit_pv(t):
                    if stage < 5:
                        return
                    et = ets[t]
                    psc = []
                    for g2 in range(2):
                        pc = pscx.tile([128, 390], F32, tag="cx", name="pc")
                        for hh in range(6):
                            h = g2 * 6 + hh
                            nc.tensor.matmul(
                                pc[:, hh * 65 : (hh + 1) * 65],
                                et[:, h * 128 : (h + 1) * 128],
                                va[:, t, h * 65 : (h + 1) * 65],
                                start=(hh == 0),
                                stop=(hh == 5),
                            )
                        psc.append(pc)
                    pscs[t] = psc

                def emit_norm_tr(t):
                    if stage < 6:
                        return
                    psc = pscs[t]
                    recip = colp.tile([128, H], F32, tag="recip", name="recip")
                    ctxn = colp.tile([128, E], F16, tag="ctxn", name="ctxn")
                    for g2 in range(2):
                        grp = psc[g2].rearrange("p (h x) -> p h x", x=65)
                        nc.vector.reciprocal(
                            recip[:, g2 * 6 : (g2 + 1) * 6].unsqueeze(2),
                            grp[:, :, 64:65],
                        )
                        nc.vector.tensor_mul(
                            ctxn[:, g2 * 384 : (g2 + 1) * 384].rearrange(
                                "p (h d) -> p h d", d=64
                            ),
                            grp[:, :, 0:64],
                            recip[:, g2 * 6 : (g2 + 1) * 6]
                            .unsqueeze(2)
                            .broadcast_to((128, 6, 64)),
                        )
                    if stage < 7:
                        return
                    pst = pscx.tile([128, NCH, 128], F16, tag="cx", name="pst")
                    for ec in range(NCH):
                        nc.tensor.transpose(
                            pst[:, ec, :],
                            ctxn[:, ec * 128 : (ec + 1) * 128],
                            ident_sb[:],
                        )
                    ctxnt = colp.tile([128, NCH, 128], F16, tag="ctxnt",
                                      name="ctxnt")
                    nc.vector.tensor_copy(ctxnt[:], pst[:])
                    ctxnts[t] = ctxnt

                def emit_wo(t, cg, store):
                    if stage < 8 or t not in store:
                        return
                    ctxnt = store.pop(t)
                    osb = colp.tile([128, E], F32, tag="osb", name="osb")
                    for half in range(2):
                        po = psmm.tile([128, 384], F32, tag="mm", name="po")
                        if with_bias:
                            nc.tensor.matmul(
                                po[:], ones_sb[:],
                                bvo_sb["bo"][:, half * 384 : (half + 1) * 384],
                                start=True, stop=False,
                            )
                        for k in range(NCH):
                            nc.tensor.matmul(
                                po[:],
                                ctxnt[:, k, :],
                                w_sb["wo"][:, k, half * 384 : (half + 1) * 384],
                                start=(k == 0 and not with_bias),
                                stop=(k == NCH - 1),
                            )
                        nc.scalar.copy(osb[:, half * 384 : (half + 1) * 384], po[:])
                    # all outputs on sync: it is ~idle, HWDGE transfers
                    # finish in ~1.1us, and the tail avoids the 3us SWDGE
                    # output-queue drain
                    nc.sync.dma_start(o_d.ap()[cg], osb[:])

                emit_scores(0)
                if BLK > 1:
                    emit_scores(1)
                for t in range(BLK):
                    emit_pv(t)
                    if t + 2 < BLK:
                        emit_scores(t + 2)
                    if t >= 1:
                        emit_wo(t - 1, b * BLK + t - 1, ctxnts)
                    elif pending_wo is not None:
                        pending_wo()           # last column of previous block
                        pending_wo = None
                    emit_norm_tr(t)
                import functools
                pending_wo = functools.partial(
                    emit_wo, BLK - 1, b * BLK + BLK - 1, ctxnts
                )

            if pending_wo is not None:
                pending_wo()
    nc.compile()
    return nc


_PROGRAMS = {}


def _get_program(with_bias: bool):
    if with_bias not in _PROGRAMS:
        _PROGRAMS[with_bias] = build_program(with_bias)
    return _PROGRAMS[with_bias]


def make_in_maps(x, self_attn_padding_mask, Wq, bq, Wk, bk, Wv, bv, Wo, bo,
                 with_bias):
    scaling = float(D) ** -0.5
    # host-flattened weights: [128, NCH*E] with w[p, c*E+e] = W[c*128+p, e]
    def wflat(W, scale=1.0):
        w = (np.asarray(W, np.float32) * scale).astype(np.float16)
        return np.ascontiguousarray(
            w.reshape(NCH, 128, E).transpose(1, 0, 2).reshape(128, NCH * E)
        )

    wq = wflat(Wq, scaling)
    wk = wflat(Wk)
    wv = wflat(Wv)
    wo = wflat(Wo)
    mask = np.asarray(self_attn_padding_mask)[0]                   # (R, C)
    madd_full = np.where(mask, 0.0, -10000.0).astype(np.float32)   # (R, C)
    xf = np.asarray(x, np.float32)[:, :, 0, :]                     # (R, C, E)
    ident = np.eye(128, dtype=np.float16)
    in_maps = []
    for i in range(NCORES):
        cs = slice(i * CLOC, (i + 1) * CLOC)
        xs = (
            xf[:, cs]
            .transpose(1, 0, 2)                # (CLOC, R, E) tok-major
            .reshape(NTOK, NCH, 128)
            .transpose(1, 2, 0)                # (NCH, 128, NTOK) = x^T chunks
        )
        # block-major: x_b[b, p, c*T+t] = xs[c, p, b*T+t]
        xb = (
            xs.reshape(NCH, 128, NBLK, T)
            .transpose(2, 1, 0, 3)
            .reshape(NBLK, 128, NCH * T)
        )
        xb = np.ascontiguousarray(xb.astype(np.float16))
        m = {
            "x": xb,
            "madd": np.ascontiguousarray(madd_full[:, cs]),
            "wq": wq, "wk": wk, "wv": wv, "wo": wo,
            "ident": ident,
        }
        if with_bias:
            m["bq"] = np.ascontiguousarray(
                (np.asarray(bq, np.float32) * scaling).reshape(NCH, 128).T
            )
            m["bk"] = np.ascontiguousarray(
                np.asarray(bk, np.float32).reshape(NCH, 128).T
            )
            m["bv"] = np.asarray(bv, np.float32).astype(np.float16).reshape(1, E)
            m["bo"] = np.asarray(bo, np.float32).astype(np.float16).reshape(1, E)
        in_maps.append(m)
    return in_maps


def assemble_output(shards):
    out = np.empty((R, C, 1, E), np.float32)
    for i in range(NCORES):
        out[:, i * CLOC : (i + 1) * CLOC, 0, :] = shards[i].transpose(1, 0, 2)
    return out


def kernel(x, self_attn_padding_mask, Wq, bq, Wk, bk, Wv, bv, Wo, bo):
    global LAST_RESULTS
    with_bias = any(
        bool(np.any(np.asarray(b))) for b in (bq, bk, bv, bo)
    )
    nc = _get_program(with_bias)
    in_maps = make_in_maps(
        x, self_attn_padding_mask, Wq, bq, Wk, bk, Wv, bv, Wo, bo, with_bias
    )
    trace = os.environ.get("KERNEL_TRACE", "") not in ("", "0")
    res = run_bass_kernel_spmd(
        nc, in_maps, core_ids=list(range(NCORES)), trace=trace
    )
    LAST_RESULTS = res
    return assemble_output([res.results[i]["o"] for i in range(NCORES)])


# revision 20
# speedup vs baseline: 1.0035x; 1.0006x over previous
"""Trainium2 Bass kernel for column self-attention (nn_ColumnSelfAttention).

Reference computation (per column c, columns are independent attention
problems):
    q = (x @ Wq + bq) * head_dim**-0.5 ; k = x @ Wk + bk ; v = x @ Wv + bv
    scores[h,c,i,j] = sum_d q[i,c,h,d] k[j,c,h,d]
    scores = where(mask[j,c], scores, -1e4); p = softmax_j(scores)
    ctx[i,c,:] = concat_h(p @ v) ; out = ctx @ Wo + bo

Sharding: the 256 columns are split across 8 NeuronCores (32 each).
Per core, tokens are ordered (column-major, row-inner) so one 128-token
tile == one column.  All matmul inputs are fp16 (fp32 PSUM accumulation);
softmax runs in fp32 on the scalar/vector engines.

Layout trick: scores are computed transposed (S_T[j,i]) so the key mask
becomes a per-partition bias fused into the Exp activation, and the
softmax denominator comes for free from an appended ones-column on V
(out[:, 64] of each head block = row sums).  The only transpose needed
is x (host-side) and the normalized context (PE transpose via identity
matmul) before the output projection.

Perf notes (vs the 400us/336us baseline):
  - weights are host-flattened to [128, NCH*E] so each is ONE contiguous
    HWDGE DMA on the sync queue (the old strided gather on the SWDGE
    queue gated the first matmul to t=16.3us); x is host-blocked to
    [NBLK, 128, NCH*T], block 0 split in two halves on the scalar queue
    (a single HWDGE queue only sustains ~90-180 GB/s, so splitting the
    critical first loads across both queues matters).
  - qz (zero-padded Q) and va (V+ones) live in persistent double
    buffers, memset ONCE (the zero/one regions are never overwritten),
    not per block: GpSimd -38us busy.
  - psmm PSUM pool bufs 2->3 (WO matmuls no longer stall ~290ns on the
    scalar PSUM->SBUF drains); pscx 3->2 to stay within 8 banks.
  - last column's output DMA goes on sync (idle by then) instead of the
    SWDGE queue, shortening the tail drain.
  - DMA xbar transposes of ctx and per-column output DMAs on sync were
    tried and REVERTED: each DMA_TRANSPOSE costs ~1.3us of sync
    sequencer time and the rotating DMA-completion semaphores couple
    the transpose path to slow SWDGE output transfers, stalling WO
    matmuls 4-5us per block (411us / 394us vs 336us baseline).
"""

import os
import numpy as np

import concourse.bacc as bacc
import concourse.tile as tile
import concourse.mybir as mybir
from concourse import bass
from concourse.bass_utils import run_bass_kernel_spmd

R, C, E, H, D = 128, 256, 768, 12, 64
NCORES = 8
CLOC = C // NCORES            # 32 columns per core
BLK = 4                       # columns per block
NBLK = CLOC // BLK
T = BLK * R                   # 512 tokens per block
NTOK = CLOC * R               # 4096 tokens per core
NCH = E // 128                # 6 chunks of the embedding dim
F16 = mybir.dt.float16
F32 = mybir.dt.float32
Act = mybir.ActivationFunctionType

LAST_RESULTS = None           # for test.py introspection


def build_program(with_bias: bool, nblocks: int = NBLK, stage: int = 8):
    nc = bacc.Bacc("TRN2", target_bir_lowering=False, debug=False)

    # x^T per core, host-pretransposed AND host-blocked:
    # x_d[b, p, c*T+t] = x^T[c*128+p, b*512+t] -- one contiguous DMA/block
    x_d = nc.dram_tensor("x", [NBLK, 128, NCH * T], F16, kind="ExternalInput")
    madd_d = nc.dram_tensor("madd", [R, CLOC], F32, kind="ExternalInput")
    ident_d = nc.dram_tensor("ident", [128, 128], F16, kind="ExternalInput")
    # weights host-flattened: w_d[p, c*E+e] = W[c*128+p, e] -- contiguous
    w_d = {
        n: nc.dram_tensor(n, [128, NCH * E], F16, kind="ExternalInput")
        for n in ("wq", "wk", "wv", "wo")
    }
    if with_bias:
        bqk_d = {
            n: nc.dram_tensor(n, [128, NCH], F32, kind="ExternalInput")
            for n in ("bq", "bk")
        }
        bvo_d = {
            n: nc.dram_tensor(n, [1, E], F16, kind="ExternalInput")
            for n in ("bv", "bo")
        }
    o_d = nc.dram_tensor("o", [nblocks * BLK, R, E], F32, kind="ExternalOutput")

    with tile.TileContext(nc) as tc:
        with (
            tc.tile_pool(name="const", bufs=1) as const,
            tc.tile_pool(name="blk", bufs=2) as blkp,
            tc.tile_pool(name="col", bufs=4) as colp,
            tc.tile_pool(name="psmm", bufs=3, space="PSUM") as psmm,
            tc.tile_pool(name="pss", bufs=3, space="PSUM") as pssp,
            tc.tile_pool(name="pscx", bufs=2, space="PSUM") as pscx,
        ):
            w_sb = {}
            for n in ("wq", "wk", "wv", "wo"):
                w_sb[n] = const.tile([128, NCH, E], F16, tag=n, name=f"w_{n}")
            madd_sb = const.tile([R, CLOC], F32, tag="madd")
            ident_sb = const.tile([128, 128], F16, tag="ident")
            # persistent double buffers: the zero rows of qz / ones cols of
            # va are written once here and never overwritten by the loop
            qzp = [const.tile([128, H, T], F16, tag=f"qz{i}", name=f"qz{i}")
                   for i in range(2)]
            vap = [const.tile([128, BLK, H * 65], F16, tag=f"va{i}",
                              name=f"va{i}") for i in range(2)]

            # head loading: a single DMA queue only sustains ~105 GB/s, so
            # the first-needed bytes (wq + block-0 x) are chunk-paired
            # across all three queues in the PE's consumption order
            # (Q-proj co=0 consumes (wq_k, xt_k) for k=0..5)
            xt0 = blkp.tile([128, NCH, T], F16, tag="xt", name="xt0")
            x0flat = xt0.rearrange("p c t -> p (c t)")
            qdma = [nc.sync, nc.scalar, nc.gpsimd]
            for k in range(NCH):
                eng = qdma[k % 3]
                eng.dma_start(
                    w_sb["wq"][:, k, :], w_d["wq"].ap()[:, k * E : (k + 1) * E]
                )
                eng.dma_start(
                    x0flat[:, k * T : (k + 1) * T],
                    x_d.ap()[0, :, k * T : (k + 1) * T],
                )
            # remaining weights: halves on sync+scalar, in consumption order
            for n in ("wk", "wv", "wo"):
                wf = w_sb[n].rearrange("p c e -> p (c e)")
                hw = NCH * E // 2
                nc.sync.dma_start(wf[:, 0:hw], w_d[n].ap()[:, 0:hw])
                nc.scalar.dma_start(wf[:, hw:], w_d[n].ap()[:, hw:])
            # gpsimd: small loads, then block-1's one-time memsets
            nc.gpsimd.dma_start(madd_sb[:], madd_d.ap())
            nc.gpsimd.dma_start(ident_sb[:], ident_d.ap())
            if with_bias:
                bqk_sb = {}
                for n in ("bq", "bk"):
                    bqk_sb[n] = const.tile([128, NCH], F32, tag=n, name=f"b_{n}")
                    nc.gpsimd.dma_start(bqk_sb[n][:], bqk_d[n].ap())
                bvo_sb = {}
                for n in ("bv", "bo"):
                    bvo_sb[n] = const.tile([1, E], F16, tag=n, name=f"b_{n}")
                    nc.gpsimd.dma_start(bvo_sb[n][:], bvo_d[n].ap())
                ones_sb = const.tile([1, 128], F16, tag="ones")
                nc.gpsimd.memset(ones_sb[:], 1.0)
            # block-0's qz/va memsets on the (idle) vector engine so they
            # are ready before the first qz copies at ~10us
            nc.vector.memset(qzp[0][:], 0.0)
            nc.vector.memset(vap[0][:], 1.0)
            nc.gpsimd.memset(qzp[1][:], 0.0)
            nc.gpsimd.memset(vap[1][:], 1.0)

            pending_wo = None
            for b in range(nblocks):
                qz = qzp[b % 2]
                va = vap[b % 2]
                # ---- x^T for this block: one contiguous DMA ----
                if b == 0:
                    xt = xt0
                else:
                    xt = blkp.tile([128, NCH, T], F16, tag="xt")
                    nc.scalar.dma_start(
                        xt.rearrange("p c t -> p (c t)"), x_d.ap()[b]
                    )

                if stage < 2:
                    continue
                # ---- Q^T, K^T projections: (e_out, tok) ----
                qt = blkp.tile([128, NCH, T], F16, tag="qt")
                kt = blkp.tile([128, NCH, T], F16, tag="kt")
                for wname, bname, dst in (("wq", "bq", qt), ("wk", "bk", kt)):
                    for co in range(NCH):
                        ps = psmm.tile([128, T], F32, tag="mm")
                        for k in range(NCH):
                            nc.tensor.matmul(
                                ps[:],
                                w_sb[wname][:, k, co * 128 : (co + 1) * 128],
                                xt[:, k, :],
                                start=(k == 0),
                                stop=(k == NCH - 1),
                            )
                        if with_bias:
                            nc.scalar.activation(
                                dst[:, co, :], ps[:], Act.Identity,
                                bias=bqk_sb[bname][:, co : co + 1],
                            )
                        elif wname == "wk":
                            # K copies on vector: scalar alone (Q copies +
                            # exp backlog) fell behind the PE during the QK
                            # phase and stalled matmuls on psmm WAR
                            nc.vector.tensor_copy(dst[:, co, :], ps[:])
                        else:
                            nc.scalar.copy(dst[:, co, :], ps[:])
                        if stage >= 3 and wname == "wq":
                            # per-head zero-padded Q^T, emitted per-chunk so
                            # the DVE copies spread across the QK phase
                            # instead of bursting at block start
                            # (base-partition-64 matmuls into shared PSUM
                            # banks crash HW; scores contract K=128 instead,
                            # with the other head's rows zeroed on the Q side)
                            for h in (2 * co, 2 * co + 1):
                                off = (h % 2) * 64
                                nc.vector.tensor_copy(
                                    qz[off : off + 64, h, :],
                                    qt[off : off + 64, co, :],
                                )

                if stage < 3:
                    continue

                # ---- V projection, natural layout, interleaved with a ones
                # column per head: va[:, t, h*65:h*65+64] = V_h, [...,64] = 1 ----
                for t in range(BLK):
                    for half in range(2):
                        psv = psmm.tile([128, 384], F32, tag="mm")
                        if with_bias:
                            nc.tensor.matmul(
                                psv[:], ones_sb[:],
                                bvo_sb["bv"][:, half * 384 : (half + 1) * 384],
                                start=True, stop=False,
                            )
                        for k in range(NCH):
                            nc.tensor.matmul(
                                psv[:],
                                xt[:, k, t * 128 : (t + 1) * 128],
                                w_sb["wv"][:, k, half * 384 : (half + 1) * 384],
                                start=(k == 0 and not with_bias),
                                stop=(k == NCH - 1),
                            )
                        dst = va[:, t, half * 390 : (half + 1) * 390]
                        dst = dst.rearrange("p (h x) -> p h x", x=65)[:, :, 0:64]
                        nc.vector.tensor_copy(
                            dst, psv[:].rearrange("p (h d) -> p h d", d=64)
                        )

                # ---- attention, software-pipelined across columns so the
                # PE always has independent work while the per-column
                # PV -> recip/normalize (DVE) -> transpose -> copy -> Wo
                # chain drains.  PE emission order per cycle t:
                #   PV(t), S(t+2), WO(t-1), TR(t)
                if stage < 4:
                    continue
                ets, pscs, ctxnts = {}, {}, {}

                def emit_scores(t):
                    cg = b * BLK + t
                    et = colp.tile([128, H * 128], F16, tag="et",
                                   name=f"et_{b}_{t}")
                    for g3 in range(3):
                        pss = pssp.tile([128, 512], F32, tag="s", name="pss")
                        for pp in range(2):
                            # head pair (2hp, 2hp+1) shares the kt chunk hp
                            # as stationary: one 256-stream matmul per pair
                            hp = g3 * 2 + pp
                            nc.tensor.matmul(
                                pss[:, pp * 256 : (pp + 1) * 256],
                                kt[:, hp, t * 128 : (t + 1) * 128],
                                qz[:, 2 * hp : 2 * hp + 2,
                                   t * 128 : (t + 1) * 128],
                                start=(pp == 0),
                                stop=(pp == 1),
                            )
                        nc.scalar.activation(
                            et[:, g3 * 512 : (g3 + 1) * 512], pss[:], Act.Exp,
                            bias=madd_sb[:, cg : cg + 1], scale=1.0,
                        )
                    ets[t] = et

                def emit_pv(t):
                    if stage < 5:
                        return
                    et = ets[t]
                    psc = []
                    for g2 in range(2):
                        pc = pscx.tile([128, 390], F32, tag="cx", name="pc")
                        for hh in range(6):
                            h = g2 * 6 + hh
                            nc.tensor.matmul(
                                pc[:, hh * 65 : (hh + 1) * 65],
                                et[:, h * 128 : (h + 1) * 128],
                                va[:, t, h * 65 : (h + 1) * 65],
                                start=(hh == 0),
                                stop=(hh == 5),
                            )
                        psc.append(pc)
                    pscs[t] = psc

                def emit_norm_tr(t):
                    if stage < 6:
                        return
                    psc = pscs[t]
                    recip = colp.tile([128, H], F32, tag="recip", name="recip")
                    ctxn = colp.tile([128, E], F16, tag="ctxn", name="ctxn")
                    for g2 in range(2):
                        grp = psc[g2].rearrange("p (h x) -> p h x", x=65)
                        nc.vector.reciprocal(
                            recip[:, g2 * 6 : (g2 + 1) * 6].unsqueeze(2),
                            grp[:, :, 64:65],
                        )
                        nc.vector.tensor_mul(
                            ctxn[:, g2 * 384 : (g2 + 1) * 384].rearrange(
                                "p (h d) -> p h d", d=64
                            ),
                            grp[:, :, 0:64],
                            recip[:, g2 * 6 : (g2 + 1) * 6]
                            .unsqueeze(2)
                            .broadcast_to((128, 6, 64)),
                        )
                    if stage < 7:
                        return
                    pst = pscx.tile([128, NCH, 128], F16, tag="cx", name="pst")
                    for ec in range(NCH):
                        nc.tensor.transpose(
                            pst[:, ec, :],
                            ctxn[:, ec * 128 : (ec + 1) * 128],
                            ident_sb[:],
                        )
                    ctxnt = colp.tile([128, NCH, 128], F16, tag="ctxnt",
                                      name="ctxnt")
                    nc.vector.tensor_copy(ctxnt[:], pst[:])
                    ctxnts[t] = ctxnt

                def emit_wo(t, cg, store):
                    if stage < 8 or t not in store:
                        return
                    ctxnt = store.pop(t)
                    osb = colp.tile([128, E], F32, tag="osb", name="osb")
                    for half in range(2):
                        po = psmm.tile([128, 384], F32, tag="mm", name="po")
                        if with_bias:
                            nc.tensor.matmul(
                                po[:], ones_sb[:],
                                bvo_sb["bo"][:, half * 384 : (half + 1) * 384],
                                start=True, stop=False,
                            )
                        for k in range(NCH):
                            nc.tensor.matmul(
                                po[:],
                                ctxnt[:, k, :],
                                w_sb["wo"][:, k, half * 384 : (half + 1) * 384],
                                start=(k == 0 and not with_bias),
                                stop=(k == NCH - 1),
                            )
                        nc.scalar.copy(osb[:, half * 384 : (half + 1) * 384], po[:])
                    if cg == nblocks * BLK - 1:
                        # sync is idle by the tail; skip the SWDGE drain
                        nc.sync.dma_start(o_d.ap()[cg], osb[:])
                    else:
                        nc.gpsimd.dma_start(o_d.ap()[cg], osb[:])

                emit_scores(0)
                if BLK > 1:
                    emit_scores(1)
                for t in range(BLK):
                    emit_pv(t)
                    if t + 2 < BLK:
                        emit_scores(t + 2)
                    if t >= 1:
                        emit_wo(t - 1, b * BLK + t - 1, ctxnts)
                    elif pending_wo is not None:
                        pending_wo()           # last column of previous block
                        pending_wo = None
                    emit_norm_tr(t)
                import functools
                pending_wo = functools.partial(
                    emit_wo, BLK - 1, b * BLK + BLK - 1, ctxnts
                )

            if pending_wo is not None:
                pending_wo()
    nc.compile()
    return nc


_PROGRAMS = {}


def _get_program(with_bias: bool):
    if with_bias not in _PROGRAMS:
        _PROGRAMS[with_bias] = build_program(with_bias)
    return _PROGRAMS[with_bias]


def make_in_maps(x, self_attn_padding_mask, Wq, bq, Wk, bk, Wv, bv, Wo, bo,
                 with_bias):
    scaling = float(D) ** -0.5
    # host-flattened weights: [128, NCH*E] with w[p, c*E+e] = W[c*128+p, e]
    def wflat(W, scale=1.0):
        w = (np.asarray(W, np.float32) * scale).astype(np.float16)
        return np.ascontiguousarray(
            w.reshape(NCH, 128, E).transpose(1, 0, 2).reshape(128, NCH * E)
        )

    wq = wflat(Wq, scaling)
    wk = wflat(Wk)
    wv = wflat(Wv)
    wo = wflat(Wo)
    mask = np.asarray(self_attn_padding_mask)[0]                   # (R, C)
    madd_full = np.where(mask, 0.0, -10000.0).astype(np.float32)   # (R, C)
    xf = np.asarray(x, np.float32)[:, :, 0, :]                     # (R, C, E)
    ident = np.eye(128, dtype=np.float16)
    in_maps = []
    for i in range(NCORES):
        cs = slice(i * CLOC, (i + 1) * CLOC)
        xs = (
            xf[:, cs]
            .transpose(1, 0, 2)                # (CLOC, R, E) tok-major
            .reshape(NTOK, NCH, 128)
            .transpose(1, 2, 0)                # (NCH, 128, NTOK) = x^T chunks
        )
        # block-major: x_b[b, p, c*T+t] = xs[c, p, b*T+t]
        xb = (
            xs.reshape(NCH, 128, NBLK, T)
            .transpose(2, 1, 0, 3)
            .reshape(NBLK, 128, NCH * T)
        )
        xb = np.ascontiguousarray(xb.astype(np.float16))
        m = {
            "x": xb,
            "madd": np.ascontiguousarray(madd_full[:, cs]),
            "wq": wq, "wk": wk, "wv": wv, "wo": wo,
            "ident": ident,
        }
        if with_bias:
            m["bq"] = np.ascontiguousarray(
                (np.asarray(bq, np.float32) * scaling).reshape(NCH, 128).T
            )
            m["bk"] = np.ascontiguousarray(
                np.asarray(bk, np.float32).reshape(NCH, 128).T
            )
            m["bv"] = np.asarray(bv, np.float32).astype(np.float16).reshape(1, E)
            m["bo"] = np.asarray(bo, np.float32).astype(np.float16).reshape(1, E)
        in_maps.append(m)
    return in_maps


def assemble_output(shards):
    out = np.empty((R, C, 1, E), np.float32)
    for i in range(NCORES):
        out[:, i * CLOC : (i + 1) * CLOC, 0, :] = shards[i].transpose(1, 0, 2)
    return out


def kernel(x, self_attn_padding_mask, Wq, bq, Wk, bk, Wv, bv, Wo, bo):
    global LAST_RESULTS
    with_bias = any(
        bool(np.any(np.asarray(b))) for b in (bq, bk, bv, bo)
    )
    nc = _get_program(with_bias)
    in_maps = make_in_maps(
        x, self_attn_padding_mask, Wq, bq, Wk, bk, Wv, bv, Wo, bo, with_bias
    )
    trace = os.environ.get("KERNEL_TRACE", "") not in ("", "0")
    res = run_bass_kernel_spmd(
        nc, in_maps, core_ids=list(range(NCORES)), trace=trace
    )
    LAST_RESULTS = res
    return assemble_output([res.results[i]["o"] for i in range(NCORES)])
